# revision 11
# baseline (speedup 1.0000x reference)
import numpy as np

N = 10000
E = 160000
T = 32
H = 256
K = 4
NF = 192
T_TEACH = 24

CORES = 8
NLOC = 1250          # nodes per core
NPAD = 1280
ECH = 2048           # edge chunk
G = 16               # grid slots per node (deg-16 grid)
NCH_G = NPAD * G // ECH      # 10 grid chunks
NCH_S = 2                    # spill chunks
NCHUNK = NCH_G + NCH_S       # 12
NT = [(0, 512), (512, 1024), (1024, 1280)]

_CACHE = {}


def _sigmoid(x):
    return 1.0 / (1.0 + np.exp(-x))


def _numpy_impl(x, edge_index, z, Wf, bf, W1, b1, W2, b2,
                Wih, bih, Whh, bhh, Wo, bo):
    x = np.asarray(x, np.float32)
    src = np.asarray(edge_index[0], np.int64)
    dst = np.asarray(edge_index[1], np.int64)
    zw = np.asarray(z, np.float32)[:, 1:K].T.copy()

    order = np.argsort(dst, kind="stable")
    dst_s = dst[order]
    src_s = src[order]
    zw_s = np.ascontiguousarray(zw[:, order])

    starts = np.searchsorted(dst_s, np.arange(N))
    deg = np.bincount(dst, minlength=N).astype(np.float32)
    cnt = np.maximum(deg, 1.0)[:, None]
    starts_c = np.minimum(starts, E - 1)
    empty = deg == 0

    def segsum(X):
        out = np.add.reduceat(X, starts_c, axis=0)
        if empty.any():
            out[empty] = 0.0
        return out

    Zk = np.stack([segsum(zw_s[k][:, None])[:, 0] for k in range(K - 1)])
    m_bias = sum(np.outer(Zk[k], b2[k]) for k in range(K - 1))

    W1a = np.ascontiguousarray(W1[:, :NF, :])
    W1b = np.ascontiguousarray(W1[:, NF:, :])

    x_seq = x.reshape(N, T, 6).transpose(1, 0, 2)
    h = np.zeros((N, H), np.float32)
    prev = np.zeros((N, 6), np.float32)
    mus = np.empty((T, N, 6), np.float32)
    WihT = Wih.T.copy()
    WhhT = Whh.T.copy()

    for t in range(T):
        inputs = x_seq[t] if t < T_TEACH else prev
        fh = np.maximum(h @ Wf + bf, 0.0)
        acc = np.zeros((N, H), np.float32)
        for k in range(K - 1):
            A = fh @ W1a[k] + b1[k]
            B = fh @ W1b[k]
            h1 = A[dst_s] + B[src_s]
            np.maximum(h1, 0.0, out=h1)
            h1 *= zw_s[k][:, None]
            acc += segsum(h1) @ W2[k]
        m = (acc + m_bias) / cnt
        gx = inputs @ WihT + bih
        gh = m @ WhhT + bhh
        r = _sigmoid(gx[:, :H] + gh[:, :H])
        zg = _sigmoid(gx[:, H:2 * H] + gh[:, H:2 * H])
        n = np.tanh(gx[:, 2 * H:] + r * gh[:, 2 * H:])
        h = (1.0 - zg) * n + zg * m
        mu = inputs + np.maximum(h @ Wo + bo, 0.0)
        mus[t] = mu
        prev = mu

    return mus.transpose(1, 0, 2).reshape(N, NF).astype(np.float32)


# ---------------- Bass device implementation ----------------

def _build_nc():
    import sys
    if "/opt/trn_rl_repo" not in sys.path:
        sys.path.insert(0, "/opt/trn_rl_repo")
    import concourse.bacc as bacc
    import concourse.mybir as mybir
    import concourse.tile as tile

    mdt = mybir.dt
    AF = mybir.ActivationFunctionType
    AL = mybir.AluOpType
    f32, bf16, i16 = mdt.float32, mdt.bfloat16, mdt.int16

    nc = bacc.Bacc(None, target_bir_lowering=False, debug=False,
                   num_devices=CORES)

    ein = lambda n_, s_, d_: nc.dram_tensor(n_, s_, d_, kind="ExternalInput")
    xT = ein("xT", [NF, NPAD], f32)
    srcw = ein("srcw", [128, NCHUNK * 128], i16)
    dstaw = ein("dstaw", [128, NCH_S * 128], i16)
    dstsw = ein("dstsw", [128, NCH_S * 128], i16)
    zwd = ein("zwd", [128, 3 * NCHUNK * ECH * 2], bf16)
    mbd = ein("mbd", [128, 2 * NPAD], f32)
    invd = ein("invd", [128, NPAD], f32)
    zbd = ein("zbd", [128, 2560], bf16)
    zfd = ein("zfd", [128, NPAD], f32)
    wfd = ein("wfd", [128, 2 * 2 * 128], bf16)
    w1ad = ein("w1ad", [128, 3 * 2 * 2 * 128], bf16)
    w1bd = ein("w1bd", [128, 3 * 2 * 2 * 128], bf16)
    w2d = ein("w2d", [128, 3 * 2 * 2 * 128], bf16)
    whhd = ein("whhd", [128, 2 * 6 * 128], f32)
    wihd = ein("wihd", [6, 768], f32)
    wod = ein("wod", [128, 2 * 6], bf16)
    gbd = ein("gbd", [128, 8], f32)
    b1d = ein("b1d", [128, 6], f32)
    bfd = ein("bfd", [128, 2], f32)
    bod = ein("bod", [6, 1], f32)
    outc = nc.dram_tensor("outc", [NF, NPAD], f32, kind="ExternalOutput")

    with tile.TileContext(nc) as tc:
        with (
            tc.tile_pool(name="dram", bufs=1, space="DRAM") as dp,
            tc.tile_pool(name="sb", bufs=1) as sb,
            tc.tile_pool(name="ps", bufs=1, space="PSUM") as pp,
        ):
            # persistent sbuf tiles
            t_wf = sb.tile([128, 2, 2, 128], bf16)
            t_w1a = sb.tile([128, 3, 2, 2, 128], bf16)
            t_w1b = sb.tile([128, 3, 2, 2, 128], bf16)
            t_w2 = sb.tile([128, 3, 2, 2, 128], bf16)
            t_whh = sb.tile([128, 2, 6, 128], f32)
            t_wih = sb.tile([6, 768], f32)
            t_wo = sb.tile([128, 2, 6], bf16)
            t_gb = sb.tile([128, 8], f32)
            t_b1 = sb.tile([128, 3, 2], f32)
            t_bf = sb.tile([128, 2], f32)
            t_bo = sb.tile([6, 1], f32)
            t_src = sb.tile([128, NCHUNK, 128], i16)
            t_dsta = sb.tile([128, NCH_S, 128], i16)
            t_dsts = sb.tile([128, NCH_S, 128], i16)
            t_mb = sb.tile([128, 2, NPAD], f32)
            t_inv = sb.tile([128, NPAD], f32)

            Bfull = sb.tile([128, N, 2], bf16)
            Bloc = sb.tile([128, NPAD, 2], bf16)
            Ak = sb.tile([128, NPAD, 2], bf16)
            fh = sb.tile([128, 2, NPAD], bf16)
            hT = [sb.tile([128, 2, NPAD], bf16, name=f"hT_{i}") for i in range(2)]
            muT = [sb.tile([6, NPAD], f32, name=f"muT_{i}") for i in range(2)]
            xin = sb.tile([6, NPAD], f32)
            m_t = sb.tile([128, 2, NPAD], f32)
            s_k = sb.tile([128, NPAD, 2], bf16)
            gA = [sb.tile([128, ECH, 2], bf16, name=f"gA_{i}") for i in range(2)]
            gB = [sb.tile([128, ECH, 2], bf16, name=f"gB_{i}") for i in range(2)]
            hs = [sb.tile([128, ECH, 2], bf16, name=f"hs_{i}") for i in range(2)]
            zt = [sb.tile([128, ECH, 2], bf16, name=f"zt_{i}") for i in range(2)]
            gB4 = [sb.tile([128, 128, G, 2], bf16, name=f"gB4_{i}")
                   for i in range(2)]
            h4t = [sb.tile([128, 128, G, 2], bf16, name=f"h4t_{i}")
                   for i in range(2)]
            zt4 = [sb.tile([128, 128, G, 2], bf16, name=f"zt4_{i}")
                   for i in range(2)]
            r_s = sb.tile([128, 2, 512], f32)
            z_s = sb.tile([128, 2, 512], f32)
            t1 = sb.tile([128, 2, 512], f32)
            t2 = sb.tile([128, 2, 512], f32)
            mur = sb.tile([6, 512], f32)

            d_ib = dp.tile([128, 2560], bf16)
            d_ob = dp.tile([CORES * 128, 2560], bf16)

            P = [pp.tile([128, 512], f32, name=f"P_{i}") for i in range(8)]
            pc = [0]

            def psum2():
                t_ = P[pc[0] % 2]
                pc[0] += 1
                return t_

            dma = nc.sync.dma_start

            # load persistent data
            dma(t_wf[:], wfd[:])
            dma(t_w1a[:], w1ad[:])
            dma(t_w1b[:], w1bd[:])
            dma(t_w2[:], w2d[:])
            dma(t_whh[:], whhd[:])
            dma(t_wih[:], wihd[:])
            dma(t_wo[:], wod[:])
            dma(t_gb[:], gbd[:])
            dma(t_b1[:], b1d[:])
            dma(t_bf[:], bfd[:])
            dma(t_bo[:], bod[:])
            dma(t_src[:], srcw[:])
            dma(t_dsta[:], dstaw[:])
            dma(t_dsts[:], dstsw[:])
            dma(t_mb[:], mbd[:])
            dma(t_inv[:], invd[:])
            dma(hT[0][:], zbd[:])
            dma(muT[0][:], zfd[0:6, :])

            mm = nc.tensor.matmul
            act = nc.scalar.activation
            tt = nc.vector.tensor_tensor
            stt = nc.vector.scalar_tensor_tensor

            for t in range(T):
                hp, hn = hT[t % 2], hT[(t + 1) % 2]
                prev, cur = muT[t % 2], muT[(t + 1) % 2]
                if t < T_TEACH:
                    dma(xin[:], xT[6 * t:6 * t + 6, :])
                    inp = xin
                else:
                    inp = prev

                # fh = relu(h @ Wf + bf)   [in slot-chunk layout]
                for so in range(2):
                    for (c0, c1) in NT:
                        w = c1 - c0
                        pt = psum2()
                        for si in range(2):
                            mm(pt[:, :w], t_wf[:, si, so, :],
                               hp[:, si, c0:c1], start=(si == 0), stop=(si == 1))
                        act(fh[:, so, c0:c1], pt[:, :w], AF.Relu,
                            bias=t_bf[:, so:so + 1])

                # edge-type loop
                for k in range(3):
                    # B_loc = fh @ W1b[k]  (interleaved), AllGather -> Bfull
                    for so in range(2):
                        for (c0, c1) in NT:
                            w = c1 - c0
                            pt = psum2()
                            for si in range(2):
                                mm(pt[:, :w], t_w1b[:, k, si, so, :],
                                   fh[:, si, c0:c1], start=(si == 0), stop=(si == 1))
                            act(Bloc[:, c0:c1, so], pt[:, :w], AF.Copy)
                    dma(d_ib[:], Bloc[:])
                    nc.gpsimd.collective_compute(
                        "AllGather", AL.bypass,
                        replica_groups=[list(range(CORES))],
                        ins=[d_ib.opt()], outs=[d_ob.opt()])
                    for r in range(CORES):
                        dma(Bfull[:, NLOC * r:NLOC * r + NLOC, :],
                            d_ob[128 * r:128 * (r + 1), 0:2 * NLOC])

                    # A_k = fh @ W1a[k] + b1[k]  (local, interleaved)
                    for so in range(2):
                        for (c0, c1) in NT:
                            w = c1 - c0
                            pt = psum2()
                            for si in range(2):
                                mm(pt[:, :w], t_w1a[:, k, si, so, :],
                                   fh[:, si, c0:c1], start=(si == 0), stop=(si == 1))
                            act(Ak[:, c0:c1, so], pt[:, :w], AF.Identity,
                                bias=t_b1[:, k, so:so + 1])

                    # grid chunks: deg-16 grid, A read via broadcast AP,
                    # tree-reduce along j writes s_k directly (no zeroing)
                    for c in range(NCH_G):
                        g4, h4_, z4_ = gB4[c % 2], h4t[c % 2], zt4[c % 2]
                        off = (k * NCHUNK + c) * ECH * 2
                        dma(z4_[:], zwd[:, off:off + ECH * 2])
                        nc.gpsimd.ap_gather(g4[:].bitcast(f32), Bfull[:].bitcast(f32),
                                            t_src[:, c, :], 128, N, 1, ECH)
                        akb = Ak[:, c * 128:(c + 1) * 128, :].unsqueeze(2) \
                            .broadcast_to([128, 128, G, 2])
                        tt(g4[:], g4[:], akb, AL.add)
                        stt(h4_[:], g4[:], 0.0, z4_[:], AL.max, AL.mult)
                        tt(h4_[:, :, 0:8, :], h4_[:, :, 0:8, :],
                           h4_[:, :, 8:16, :], AL.add)
                        tt(h4_[:, :, 0:4, :], h4_[:, :, 0:4, :],
                           h4_[:, :, 4:8, :], AL.add)
                        tt(h4_[:, :, 0:2, :], h4_[:, :, 0:2, :],
                           h4_[:, :, 2:4, :], AL.add)
                        tt(s_k[:, c * 128:(c + 1) * 128, :],
                           h4_[:, :, 0, :], h4_[:, :, 1, :], AL.add)
                    # spill chunks: leftover edges (deg > 16), baseline path
                    for c in range(NCH_S):
                        cc = NCH_G + c
                        ga, gb_, h_, z_ = gA[c % 2], gB[c % 2], hs[c % 2], zt[c % 2]
                        off = (k * NCHUNK + cc) * ECH * 2
                        dma(z_[:], zwd[:, off:off + ECH * 2])
                        nc.gpsimd.ap_gather(ga[:].bitcast(f32), Ak[:].bitcast(f32),
                                            t_dsta[:, c, :], 128, NPAD, 1, ECH)
                        nc.gpsimd.ap_gather(gb_[:].bitcast(f32), Bfull[:].bitcast(f32),
                                            t_src[:, cc, :], 128, N, 1, ECH)
                        tt(ga[:], ga[:], gb_[:], AL.add)
                        stt(h_[:], ga[:], 0.0, z_[:], AL.max, AL.mult)
                        nc.gpsimd.scatter_add(s_k[:], t_dsts[:, c, :], h_[:],
                                              128, NPAD, 2, ECH)

                    # m accumulation: psum[2..8) held across k
                    for so in range(2):
                        for j, (c0, c1) in enumerate(NT):
                            w = c1 - c0
                            pt = P[2 + so * 3 + j]
                            for si in range(2):
                                mm(pt[:, :w], t_w2[:, k, si, so, :],
                                   s_k[:, c0:c1, si],
                                   start=(k == 0 and si == 0),
                                   stop=(k == 2 and si == 1))

                # m = acc * inv + m_bias_pre
                for so in range(2):
                    for j, (c0, c1) in enumerate(NT):
                        w = c1 - c0
                        pt = P[2 + so * 3 + j]
                        tt(m_t[:, so, c0:c1], pt[:, :w], t_inv[:, c0:c1], AL.mult)
                        tt(m_t[:, so, c0:c1], m_t[:, so, c0:c1],
                           t_mb[:, so, c0:c1], AL.add)

                # GRU + mu per node tile
                for j, (c0, c1) in enumerate(NT):
                    w = c1 - c0
                    for so in range(2):
                        pr, pz = P[so], P[2 + so]
                        pnx, pnh = P[4 + so], P[6 + so]
                        for g, pt in ((0, pr), (1, pz)):
                            mm(pt[:, :w],
                               t_wih[:, g * 256 + so * 128:g * 256 + so * 128 + 128],
                               inp[:, c0:c1], start=True, stop=False)
                            for si in range(2):
                                mm(pt[:, :w], t_whh[:, si, g * 2 + so, :],
                                   m_t[:, si, c0:c1], start=False, stop=(si == 1))
                        mm(pnx[:, :w],
                           t_wih[:, 512 + so * 128:512 + so * 128 + 128],
                           inp[:, c0:c1], start=True, stop=True)
                        for si in range(2):
                            mm(pnh[:, :w], t_whh[:, si, 4 + so, :],
                               m_t[:, si, c0:c1], start=(si == 0), stop=(si == 1))
                        act(r_s[:, so, :w], pr[:, :w], AF.Sigmoid,
                            bias=t_gb[:, 0 + so:1 + so])
                        act(z_s[:, so, :w], pz[:, :w], AF.Sigmoid,
                            bias=t_gb[:, 2 + so:3 + so])
                        act(t1[:, so, :w], pnh[:, :w], AF.Identity,
                            bias=t_gb[:, 6 + so:7 + so])
                        tt(t1[:, so, :w], t1[:, so, :w], r_s[:, so, :w], AL.mult)
                        act(t2[:, so, :w], pnx[:, :w], AF.Identity,
                            bias=t_gb[:, 4 + so:5 + so])
                        tt(t2[:, so, :w], t2[:, so, :w], t1[:, so, :w], AL.add)
                        act(t1[:, so, :w], t2[:, so, :w], AF.Tanh)
                        tt(t2[:, so, :w], m_t[:, so, c0:c1], t1[:, so, :w],
                           AL.subtract)
                        tt(t2[:, so, :w], t2[:, so, :w], z_s[:, so, :w], AL.mult)
                        tt(t2[:, so, :w], t2[:, so, :w], t1[:, so, :w], AL.add)
                        act(hn[:, so, c0:c1], t2[:, so, :w], AF.Copy)
                    # mu = inp + relu(h @ Wo + bo)
                    pm = P[j % 2]
                    for si in range(2):
                        mm(pm[:6, :w], t_wo[:, si, :], hn[:, si, c0:c1],
                           start=(si == 0), stop=(si == 1))
                    act(mur[:, :w], pm[:6, :w], AF.Relu, bias=t_bo[:, 0:1])
                    tt(cur[:, c0:c1], mur[:, :w], inp[:, c0:c1], AL.add)
                dma(outc[6 * t:6 * t + 6, :], cur[:])

    nc.compile()
    return nc


def _wrap16(a):
    w = np.asarray(a, np.int16).reshape(ECH // 16, 16).T
    return np.tile(w, (8, 1))


def _prep_inputs(x, edge_index, z, Wf, bf, W1, b1, W2, b2,
                 Wih, bih, Whh, bhh, Wo, bo):
    import ml_dtypes
    bft = ml_dtypes.bfloat16
    x = np.asarray(x, np.float32)
    src = np.asarray(edge_index[0], np.int64)
    dst = np.asarray(edge_index[1], np.int64)
    zw = np.asarray(z, np.float32)[:, 1:K].T.copy()

    W1a = np.zeros((3, 256, 256), np.float32)
    W1b = np.zeros((3, 256, 256), np.float32)
    W1a[:, :NF, :] = W1[:, :NF, :]
    W1b[:, :NF, :] = W1[:, NF:, :]
    Wfp = np.zeros((256, 256), np.float32)
    Wfp[:, :NF] = Wf

    def til(Wm, dt):  # [256,256] -> [128, 2, 2, 128]
        r = Wm.reshape(2, 128, 2, 128).transpose(1, 0, 2, 3)
        return np.ascontiguousarray(r).astype(dt)

    wf_t = til(Wfp, bft).reshape(128, -1)
    w1a_t = np.stack([til(W1a[k], bft) for k in range(3)], 1).reshape(128, -1)
    w1b_t = np.stack([til(W1b[k], bft) for k in range(3)], 1).reshape(128, -1)
    w2_t = np.stack([til(W2[k], bft) for k in range(3)], 1).reshape(128, -1)
    WhhT = Whh.T.astype(np.float32)  # [256, 768]
    whh_t = WhhT.reshape(2, 128, 6, 128).transpose(1, 0, 2, 3)
    whh_t = np.ascontiguousarray(whh_t).reshape(128, -1)
    wih_t = Wih.T.astype(np.float32)  # [6, 768]
    wo_t = Wo.reshape(2, 128, 6).transpose(1, 0, 2)
    wo_t = np.ascontiguousarray(wo_t).astype(bft).reshape(128, -1)

    bc = (bih + bhh).astype(np.float32)
    gb = np.zeros((128, 8), np.float32)
    for so in range(2):
        gb[:, 0 + so] = bc[0 + so * 128:128 + so * 128]
        gb[:, 2 + so] = bc[256 + so * 128:256 + 128 + so * 128]
        gb[:, 4 + so] = bih[512 + so * 128:512 + 128 + so * 128]
        gb[:, 6 + so] = bhh[512 + so * 128:512 + 128 + so * 128]
    b1t = np.zeros((128, 6), np.float32)
    for k_ in range(3):
        for so in range(2):
            b1t[:, k_ * 2 + so] = b1[k_, so * 128:so * 128 + 128]
    b1t = b1t.reshape(128, 3, 2).reshape(128, -1)
    bft_b = np.zeros((128, 2), np.float32)
    bft_b[:, 0] = np.concatenate([bf, np.zeros(128 - (NF - 128), np.float32)])[:128] \
        if False else np.pad(bf, (0, 64))[:128]
    bfp = np.pad(bf.astype(np.float32), (0, 256 - NF))
    bft_b[:, 0] = bfp[:128]
    bft_b[:, 1] = bfp[128:]
    bo_t = bo.astype(np.float32).reshape(6, 1)

    zeros_b = np.zeros((128, 2560), bft)
    zeros_f = np.zeros((128, NPAD), np.float32)

    ins = []
    for r in range(CORES):
        lo = NLOC * r
        sel = np.nonzero((dst >= lo) & (dst < lo + NLOC))[0]
        sel = sel[np.argsort(dst[sel], kind="stable")]
        ne = len(sel)
        se = src[sel]
        de = dst[sel] - lo
        zwe = zw[:, sel]

        # grid: first G edges of each dst node at slot d*G + rank
        counts = np.bincount(de, minlength=NPAD)
        starts = np.concatenate([[0], np.cumsum(counts)[:-1]])
        rank = np.arange(ne) - starts[de]
        is_grid = rank < G
        gs = de * G + rank
        GSLOT = NPAD * G
        g_src = np.zeros(GSLOT, np.int64)
        zw_g = np.zeros((3, GSLOT), np.float32)
        g_src[gs[is_grid]] = se[is_grid]
        zw_g[:, gs[is_grid]] = zwe[:, is_grid]

        sp = ~is_grid
        nsp = int(sp.sum())
        SSLOT = NCH_S * ECH
        assert nsp <= SSLOT, nsp
        s_src = np.zeros(SSLOT, np.int64)
        s_dsta = np.zeros(SSLOT, np.int64)
        s_dsts = np.full(SSLOT, -1, np.int64)
        zw_s = np.zeros((3, SSLOT), np.float32)
        s_src[:nsp] = se[sp]
        s_dsta[:nsp] = de[sp]
        s_dsts[:nsp] = de[sp]
        zw_s[:, :nsp] = zwe[:, sp]

        srcw = np.stack(
            [_wrap16(g_src[c * ECH:(c + 1) * ECH]) for c in range(NCH_G)]
            + [_wrap16(s_src[c * ECH:(c + 1) * ECH]) for c in range(NCH_S)],
            1).reshape(128, -1)
        dstaw = np.stack([_wrap16(s_dsta[c * ECH:(c + 1) * ECH])
                          for c in range(NCH_S)], 1).reshape(128, -1)
        dstsw = np.stack([_wrap16(s_dsts[c * ECH:(c + 1) * ECH])
                          for c in range(NCH_S)], 1).reshape(128, -1)

        zwall = np.concatenate([zw_g.reshape(3, NCH_G, ECH),
                                zw_s.reshape(3, NCH_S, ECH)], axis=1)
        zwrep = np.repeat(zwall, 2, axis=2)  # [3,NCHUNK,ECH*2]
        zwd = np.broadcast_to(zwrep.reshape(1, -1), (128, 3 * NCHUNK * ECH * 2))
        zwd = np.ascontiguousarray(zwd).astype(bft)

        deg = np.bincount(de, minlength=NPAD).astype(np.float32)
        cntl = np.maximum(deg, 1.0)
        inv = np.broadcast_to(1.0 / cntl, (128, NPAD)).copy().astype(np.float32)
        Zk = np.stack([np.bincount(de, weights=zwe[k_], minlength=NPAD)
                       for k_ in range(3)]).astype(np.float32)
        mb = sum(np.outer(b2[k_], Zk[k_]) for k_ in range(3))  # [256, NPAD]
        mb = mb / cntl[None, :]
        mbT = mb.reshape(2, 128, NPAD).transpose(1, 0, 2)
        mbT = np.ascontiguousarray(mbT).reshape(128, -1).astype(np.float32)

        xl = np.zeros((NF, NPAD), np.float32)
        xl[:, :NLOC] = x[lo:lo + NLOC].T

        ins.append({
            "xT": xl, "srcw": srcw, "dstaw": dstaw, "dstsw": dstsw,
            "zwd": zwd, "mbd": mbT, "invd": inv,
            "zbd": zeros_b, "zfd": zeros_f,
            "wfd": wf_t, "w1ad": w1a_t, "w1bd": w1b_t, "w2d": w2_t,
            "whhd": whh_t, "wihd": wih_t, "wod": wo_t,
            "gbd": gb, "b1d": b1t, "bfd": bft_b, "bod": bo_t,
        })
    return ins


def _bass_impl(**inputs):
    import sys
    if "/opt/trn_rl_repo" not in sys.path:
        sys.path.insert(0, "/opt/trn_rl_repo")
    from concourse.bass_utils import run_bass_kernel_spmd

    if "nc" not in _CACHE:
        _CACHE["nc"] = _build_nc()
    nc = _CACHE["nc"]
    ins = _prep_inputs(**inputs)
    import os
    trace = bool(os.environ.get("KTRACE"))
    res = run_bass_kernel_spmd(nc, ins, core_ids=list(range(CORES)),
                               trace=trace)
    if getattr(res, "exec_time_ns", None):
        globals()["LAST_EXEC_NS"] = float(res.exec_time_ns)
    full = np.empty((N, NF), np.float32)
    for r in range(CORES):
        oc = np.asarray(res.results[r]["outc"], np.float32)
        full[NLOC * r:NLOC * (r + 1), :] = oc[:, :NLOC].T
    return full


def kernel(**inputs):
    try:
        return _bass_impl(**inputs)
    except Exception as e:
        import traceback
        traceback.print_exc()
        print(f"[kernel] bass path failed ({e!r}); numpy fallback", flush=True)
        return _numpy_impl(**inputs)

